# revision 1
# baseline (speedup 1.0000x reference)
"""Causal self-attention on 8 Trainium2 NeuronCores (Bass/Tile).

Problem: x[4,2048,1024] @ W_attn[1024,3072] + b_attn -> qkv; 16-head causal
attention; y @ W_proj[1024,1024] + b_proj.

Sharding: 2D over (batch, head-group). Core c = (b = c//2, g = c%2); each
core computes q/k/v for its 8 heads over its batch, flash-style causal
attention (no max subtraction — logits are small — with the softmax
denominator accumulated as a 65th "ones" column of v), then a partial
output projection with its 512-row slice of W_proj. Host adds the two
partials per batch plus b_proj.

Matmul dtypes: qkv + scores run float32r (full-rate fp32); attention-weights
/ v / y / W_proj run bf16 (full-rate, fp32 PSUM accumulation).
"""

import numpy as np

import concourse.bass as bass
import concourse.mybir as mybir
import concourse.tile as tile
from concourse import bacc
from concourse.masks import make_identity
from concourse.bass_utils import run_bass_kernel_spmd

F32 = mybir.dt.float32
F32R = mybir.dt.float32r
BF16 = mybir.dt.bfloat16

B, T, D, H = 4, 2048, 1024, 16
HD = D // H               # 64
N_GROUPS = 2
FQ = D // N_GROUPS        # 512 features (8 heads) per core
N_CORES = B * N_GROUPS

# set by test harness to collect an NTFF trace / HW exec time
TRACE = False
LAST_RESULTS = None


def build_nc(T=T, D=D, FQ=FQ, HD=HD, qk_mm=F32R, av_dt=BF16, pj_dt=BF16, reps=1,
             ps_bufs=4, psy_bufs=2, est_bufs=4, xin_bufs=3, wqk_bufs=3, wv_bufs=3,
             skip=(), merge_exp=False, resident_w=False):
    HLOC = FQ // HD
    P = 128
    DCH = D // P
    NTB = T // P
    TCH = 512
    NTC = T // TCH
    NFB = FQ // P
    QCH = 512
    NQC = T // QCH
    JPQ = QCH // P
    HPB = P // HD
    DOUT_CH = 512
    NDOUT = D // DOUT_CH
    NLC = FQ // P
    scale = 1.0 / float(np.sqrt(HD))

    nc = bacc.Bacc()
    xb = nc.dram_tensor("xb", [T, D], F32, kind="ExternalInput")
    wq = nc.dram_tensor("wq", [D, FQ], qk_mm, kind="ExternalInput")
    wk = nc.dram_tensor("wk", [D, FQ], qk_mm, kind="ExternalInput")
    wv = nc.dram_tensor("wv", [D, FQ], qk_mm, kind="ExternalInput")
    bq = nc.dram_tensor("bq", [FQ], F32, kind="ExternalInput")
    bk = nc.dram_tensor("bk", [FQ], F32, kind="ExternalInput")
    bv = nc.dram_tensor("bv", [FQ], qk_mm, kind="ExternalInput")
    wp = nc.dram_tensor("wp", [FQ, D], F32, kind="ExternalInput")
    out = nc.dram_tensor("out", [T, D], F32, kind="ExternalOutput")

    with tile.TileContext(nc) as tc:
        with (
            tc.tile_pool(name="const", bufs=1) as const,
            tc.tile_pool(name="big", bufs=1) as big,
            tc.tile_pool(name="xin", bufs=xin_bufs) as xin,
            tc.tile_pool(name="xtp", bufs=2) as xtp,
            tc.tile_pool(name="wqk", bufs=wqk_bufs) as wqkp,
            tc.tile_pool(name="wvp", bufs=wv_bufs) as wvp,
            tc.tile_pool(name="est", bufs=est_bufs) as est,
            tc.tile_pool(name="small", bufs=3) as small,
            tc.tile_pool(name="outp", bufs=3) as outp,
            tc.tile_pool(name="ps", bufs=(2 if merge_exp else ps_bufs), space="PSUM") as ps,
            tc.tile_pool(name="psy", bufs=psy_bufs, space="PSUM") as psy,
            tc.tile_pool(name="psc", bufs=1, space="PSUM") as psc,
        ):
            ident = const.tile([P, P], F32)
            make_identity(nc, ident)
            ones_f32 = const.tile([1, P], F32, tag="ones_f32")
            nc.vector.memset(ones_f32, 1.0)
            ones_row = const.tile([1, P], qk_mm)
            nc.vector.tensor_copy(out=ones_row, in_=ones_f32)
            # diagonal-block masks: mask_r[p, f] = 1 if f >= p + P*r else 0
            masks = []
            for r in range(JPQ if not merge_exp else 0):
                m = const.tile([P, QCH], BF16, tag=f"mask{r}")
                nc.gpsimd.memset(m, 1.0)
                nc.gpsimd.affine_select(
                    out=m, in_=m,
                    compare_op=mybir.AluOpType.is_ge,
                    fill=0.0,
                    base=-P * r,
                    pattern=[[1, QCH]],
                    channel_multiplier=-1,
                )
                masks.append(m)
            bq_sb = const.tile([P, NFB], F32, tag="bq")
            nc.sync.dma_start(out=bq_sb, in_=bq.rearrange("(o p) -> p o", p=P))
            bk_sb = const.tile([P, NFB], F32, tag="bk")
            nc.sync.dma_start(out=bk_sb, in_=bk.rearrange("(o p) -> p o", p=P))
            bv_sb = const.tile([1, FQ], qk_mm)
            nc.sync.dma_start(out=bv_sb, in_=bv[None, :])

            wp_sb = big.tile([P, NLC * NDOUT, DOUT_CH], pj_dt, tag="wp")
            for i in range(NLC):
                stage = wvp.tile([P, D], F32, tag="wpstage")
                nc.sync.dma_start(out=stage, in_=wp[i * P:(i + 1) * P, :])
                for o in range(NDOUT):
                    nc.vector.tensor_copy(
                        out=wp_sb[:, i * NDOUT + o, :],
                        in_=stage[:, o * DOUT_CH:(o + 1) * DOUT_CH],
                    )

            if resident_w:
                wq_sb = big.tile([P, DCH, FQ], qk_mm, tag="wq_sb")
                nc.sync.dma_start(
                    out=wq_sb, in_=wq.rearrange("(dc p) f -> p dc f", p=P))
                wk_sb = big.tile([P, DCH, FQ], qk_mm, tag="wk_sb")
                nc.sync.dma_start(
                    out=wk_sb, in_=wk.rearrange("(dc p) f -> p dc f", p=P))
                wv_sb = big.tile([P, DCH, FQ], qk_mm, tag="wv_sb")
                nc.sync.dma_start(out=wv_sb, in_=wv.rearrange("(dc p) f -> p dc f", p=P))

            for _rep in range(reps):
              qT = big.tile([P, NFB, T], qk_mm, tag="qT")       # [f%128, fb, tok]
              kT = big.tile([P, NFB, T], qk_mm, tag="kT")
              v_aug = big.tile([P, NTB, HLOC, HD + 1], av_dt, tag="v")
              yT = big.tile([P, NLC, T], pj_dt, tag="yT")     # [dloc%128, lc, tok]

              nc.vector.memset(v_aug[:, :, :, HD:HD + 1], 1.0)

              # stage A+B: transpose x, project q/k (-> [f, tok]) and v (-> [tok, f])
              for tch in range(NTC):
                  t0 = tch * TCH
                  xT = xtp.tile([P, DCH, TCH], qk_mm, tag="xT")
                  for tb in range(TCH // P):
                      x_tile = xin.tile([P, D], F32, tag="xin")
                      nc.sync.dma_start(
                          out=x_tile, in_=xb[t0 + tb * P: t0 + (tb + 1) * P, :])
                      for d4 in range(DCH // 4):
                          pst = ps.tile([P, 512], F32, tag="ps")
                          for dd in range(4):
                              d = d4 * 4 + dd
                              nc.tensor.transpose(
                                  pst[:, dd * P:(dd + 1) * P],
                                  x_tile[:, d * P:(d + 1) * P], ident)
                          nc.vector.tensor_copy(
                              out=xT[:, d4 * 4:(d4 + 1) * 4, tb * P:(tb + 1) * P],
                              in_=pst.rearrange("p (dd q) -> p dd q", q=P))
                  for (w_dram, bias_sb, dstT, w_res) in (
                          (wq, bq_sb, qT, "q"), (wk, bk_sb, kT, "k")):
                      for fb in range(NFB):
                          if resident_w:
                              wt = (wq_sb if w_res == "q" else wk_sb)[
                                  :, :, fb * P:(fb + 1) * P]
                          else:
                              wt = wqkp.tile([P, DCH, P], qk_mm, tag="wqk")
                              nc.sync.dma_start(
                                  out=wt,
                                  in_=w_dram.rearrange("(dc p) f -> p dc f", p=P)[
                                      :, :, fb * P:(fb + 1) * P],
                              )
                          pq = ps.tile([P, 512], F32, tag="ps")
                          for d in range(DCH):
                              nc.tensor.matmul(
                                  pq[:, :TCH],
                                  wt[:, d, :],
                                  xT[:, d, :],
                                  start=(d == 0), stop=(d == DCH - 1),
                              )
                          nc.vector.tensor_scalar_add(
                              out=dstT[:, fb, t0:t0 + TCH], in0=pq[:, :TCH],
                              scalar1=bias_sb[:, fb:fb + 1],
                          )
                  for tb in range(TCH // P):
                      pv = ps.tile([P, 512], F32, tag="ps")
                      for d in range(DCH):
                          if resident_w:
                              wvt = wv_sb[:, d, :]
                          else:
                              wvt = wvp.tile([P, FQ], qk_mm, tag="wv")
                              nc.sync.dma_start(out=wvt, in_=wv[d * P:(d + 1) * P, :])
                          nc.tensor.matmul(
                              pv[:, :FQ],
                              xT[:, d, tb * P:(tb + 1) * P],
                              wvt,
                              start=(d == 0), stop=False,
                          )
                      nc.tensor.matmul(
                          pv[:, :FQ],
                          ones_row,
                          bv_sb,
                          start=False, stop=True,
                      )
                      tbg = tch * (TCH // P) + tb
                      nc.vector.tensor_copy(
                          out=v_aug[:, tbg, :, 0:HD],
                          in_=pv[:, :FQ].rearrange("p (h d) -> p h d", d=HD),
                      )

              # stage C: causal attention per head; denominator rides as row HD
              if merge_exp:
                mask_cat = const.tile([P, 4 * QCH], BF16, tag="mask_cat")
                for r in range(JPQ):
                    nc.gpsimd.memset(mask_cat[:, r * QCH:(r + 1) * QCH], 1.0)
                    nc.gpsimd.affine_select(
                        out=mask_cat[:, r * QCH:(r + 1) * QCH],
                        in_=mask_cat[:, r * QCH:(r + 1) * QCH],
                        compare_op=mybir.AluOpType.is_ge,
                        fill=0.0,
                        base=-P * r,
                        pattern=[[1, QCH]],
                        channel_multiplier=-1,
                    )
                for h in range(HLOC):
                    fb = h // HPB
                    p0 = (h % HPB) * HD
                    for c in range(NQC):
                        q0 = c * QCH
                        py = psy.tile([P, 512], F32, tag="psy")
                        ngrp = c + 1
                        for g in range(ngrp):
                            pstc = psc.tile([P, 4 * QCH], F32, tag="psc")
                            for jj in range(JPQ):
                                j = g * JPQ + jj
                                nc.tensor.matmul(
                                    pstc[:, jj * QCH:(jj + 1) * QCH],
                                    kT[p0:p0 + HD, fb, j * P:(j + 1) * P],
                                    qT[p0:p0 + HD, fb, q0:q0 + QCH],
                                    start=True, stop=True,
                                )
                            eb = est.tile([P, 4 * QCH], av_dt, tag="est")
                            nc.scalar.activation(
                                out=eb, in_=pstc,
                                func=mybir.ActivationFunctionType.Exp,
                                scale=scale,
                            )
                            if g == ngrp - 1:
                                nc.vector.tensor_mul(out=eb, in0=eb, in1=mask_cat)
                            for jj in range(JPQ):
                                j = g * JPQ + jj
                                nc.tensor.matmul(
                                    py[:HD + 1, :QCH],
                                    v_aug[:, j, h, :],
                                    eb[:, jj * QCH:(jj + 1) * QCH],
                                    start=(j == 0), stop=(j == JPQ * ngrp - 1),
                                )
                        recip = small.tile([1, QCH], F32, tag="recip")
                        nc.vector.reciprocal(out=recip, in_=py[HD:HD + 1, :QCH])
                        bcast = small.tile([HD, QCH], F32, tag="bcast")
                        nc.gpsimd.partition_broadcast(bcast, recip)
                        nc.vector.tensor_mul(
                            out=yT[p0:p0 + HD, fb, q0:q0 + QCH],
                            in0=py[:HD, :QCH],
                            in1=bcast,
                        )
              for h in range(HLOC if not merge_exp else 0):
                  fb = h // HPB
                  p0 = (h % HPB) * HD
                  for c in range(NQC):
                      q0 = c * QCH
                      py = psy.tile([P, 512], F32, tag="psy")
                      nj = JPQ * c + JPQ
                      for j in range(nj):
                          pst = ps.tile([P, 512], F32, tag="ps")
                          nc.tensor.matmul(
                              pst[:, :QCH],
                              kT[p0:p0 + HD, fb, j * P:(j + 1) * P],
                              qT[p0:p0 + HD, fb, q0:q0 + QCH],
                              start=True, stop=True,
                          )
                          e = est.tile([P, QCH], av_dt, tag="est")
                          nc.scalar.activation(
                              out=e, in_=pst[:, :QCH],
                              func=mybir.ActivationFunctionType.Exp,
                              scale=scale,
                          )
                          r = j - JPQ * c
                          if r >= 0:
                              nc.vector.tensor_mul(out=e, in0=e, in1=masks[r])
                          nc.tensor.matmul(
                              py[:HD + 1, :QCH],
                              v_aug[:, j, h, :],
                              e,
                              start=(j == 0), stop=(j == nj - 1),
                          )
                      recip = small.tile([1, QCH], F32, tag="recip")
                      nc.vector.reciprocal(out=recip, in_=py[HD:HD + 1, :QCH])
                      bcast = small.tile([HD, QCH], F32, tag="bcast")
                      nc.gpsimd.partition_broadcast(bcast, recip)
                      nc.vector.tensor_mul(
                          out=yT[p0:p0 + HD, fb, q0:q0 + QCH],
                          in0=py[:HD, :QCH],
                          in1=bcast,
                      )

              # stage D: partial output projection (host adds b_proj)
              for tb in range(NTB):
                  for o in range(NDOUT):
                      po = ps.tile([P, 512], F32, tag="ps")
                      for i in range(NLC):
                          nc.tensor.matmul(
                              po[:, :DOUT_CH],
                              yT[:, i, tb * P:(tb + 1) * P],
                              wp_sb[:, i * NDOUT + o, :],
                              start=(i == 0), stop=(i == NLC - 1),
                          )
                      ot = outp.tile([P, DOUT_CH], F32, tag="out")
                      nc.vector.tensor_copy(out=ot, in_=po[:, :DOUT_CH])
                      nc.sync.dma_start(
                          out=out[tb * P:(tb + 1) * P, o * DOUT_CH:(o + 1) * DOUT_CH],
                          in_=ot,
                      )

    nc.finalize()
    return nc


# default build configuration used by kernel(); _core_inputs casts the
# weight inputs to match QK_DT.
DEFAULT_CFG = dict()
QK_DT = F32R

_NC_CACHE = {}


def _get_nc():
    if "nc" not in _NC_CACHE:
        _NC_CACHE["nc"] = build_nc(**DEFAULT_CFG)
    return _NC_CACHE["nc"]


def _core_inputs(inputs):
    x = np.ascontiguousarray(np.asarray(inputs["x"], dtype=np.float32))
    W = np.asarray(inputs["W_attn"], dtype=np.float32)
    ba = np.asarray(inputs["b_attn"], dtype=np.float32)
    Wp = np.asarray(inputs["W_proj"], dtype=np.float32)
    if QK_DT == BF16:
        import ml_dtypes
        wdt = ml_dtypes.bfloat16
    else:
        wdt = np.float32
    maps = []
    for c in range(N_CORES):
        b, g = c // N_GROUPS, c % N_GROUPS
        s = slice(g * FQ, (g + 1) * FQ)
        maps.append({
            "xb": np.ascontiguousarray(x[b]),
            "wq": np.ascontiguousarray(W[:, 0:D][:, s]).astype(wdt),
            "wk": np.ascontiguousarray(W[:, D:2 * D][:, s]).astype(wdt),
            "wv": np.ascontiguousarray(W[:, 2 * D:3 * D][:, s]).astype(wdt),
            "bq": np.ascontiguousarray(ba[0:D][s]),
            "bk": np.ascontiguousarray(ba[D:2 * D][s]),
            "bv": np.ascontiguousarray(ba[2 * D:3 * D][s]).astype(wdt),
            "wp": np.ascontiguousarray(Wp[s, :]),
        })
    return maps


def kernel(**inputs) -> np.ndarray:
    global LAST_RESULTS
    nc = _get_nc()
    maps = _core_inputs(inputs)
    res = run_bass_kernel_spmd(
        nc, maps, list(range(N_CORES)), trace=TRACE,
        trace_cores=list(range(N_CORES)) if TRACE else None,
    )
    LAST_RESULTS = res
    bp = np.asarray(inputs["b_proj"], dtype=np.float32)
    out = np.empty((B, T, D), dtype=np.float32)
    for b in range(B):
        acc = res.results[b * N_GROUPS]["out"].astype(np.float32).copy()
        for g in range(1, N_GROUPS):
            acc += res.results[b * N_GROUPS + g]["out"]
        out[b] = acc + bp
    return out

